# revision 12
# baseline (speedup 1.0000x reference)
"""Causal self-attention (B=2, T=4096, C=768, H=12, D=64) on 8 trn2 cores.

Sharding: (B, H) -> 24 (batch, head) pairs, 3 heads per core.
Core c handles batch b = c // 4 and heads 3*(c%4) .. 3*(c%4)+2.
Each core computes the qkv projection for its heads, flash-style causal
attention (no running max; unnormalized P = exp(s/8), row sums via an
appended ones-column on V), and its partial output projection
(contraction over its 192 attn-output channels). The host sums the 4
partials per batch and adds the bias.

v2: software-pipelined emission to keep the PE continuously busy (HAM
at K=8/8): per query block tb we emit qkv(tb), attention(tb), then the
normalize+project stage of tb-1 (whose DVE reciprocal/copies completed
during attention(tb)'s span). Diagonal 512-blocks are trimmed at 128
granularity (scores/exp/mask/PV skip fully-masked 128x128 sub-blocks).
Phase-1 PSUM->SBUF copies run on DVE, keeping ACT exclusively on exp.

Per-core layouts:
  xT   [768, 4096]   x[b].T so the qk projection streams tokens in the free dim
  wqk  [768, 3, 128] col groups: [Wq_h0|Wq_h1], [Wk_h0|Wk_h1], [Wq_h2|Wk_h2]
  wvp  [768, 256]    [Wv_h0 Wv_h1 Wv_h2 0]
  wp   [3, 64, 768]  Wproj row chunk per head
  consts [128, 544]  tri mask [128,128], Z mask [128,256], ones row, v-ones
Heads 0/1 are row-paired on the PE (head 0 in partitions 0-63, head 1 in
64-127) for the D=64-contraction score matmuls; head 2 runs solo in
partitions 0-63. Scores are computed transposed (ST[k, q]) so the PV
matmul contracts k on the partition dim with V in natural [t, d] layout.
"""

import numpy as np

B, T, C, H, D = 2, 4096, 768, 12, 64
HPC = 3          # heads per core
NCORES = 8
QB = 512         # query block (psum bank width in fp32)
NQB = T // QB    # 8
KT = 128         # key tile
VSTRIDE = 200    # per-k-tile column stride in vbig (3*65 used + 5 pad)

_COMPILED = {}
TRIM = True


def _qb_entries(qb):
    """Per-group score/PV work items for query block qb.

    Each group is a list of (ktile, col_off, width, q_off, mask) where
    mask is None, ("tri", col), ("z", col) or ("m4", col, tt). col_off
    is the column offset inside the group's st/pt tile; q_off the query
    offset inside the 512-query block.
    """
    k0 = 4 * qb
    groups = []
    if not TRIM:
        for g in range(2 * qb + 2):
            ents = []
            for i in range(2):
                kt = 2 * g + i
                m = ("m4", 512 * i, kt - k0) if kt >= k0 else None
                ents.append((kt, 512 * i, 512, 0, m))
            groups.append(ents)
        return groups
    for g in range(2 * qb):
        groups.append([(2 * g, 0, 512, 0, None), (2 * g + 1, 512, 512, 0, None)])
    groups.append(
        [(k0, 0, 512, 0, ("tri", 0)), (k0 + 1, 512, 384, 128, ("tri", 512))]
    )
    groups.append(
        [(k0 + 2, 0, 256, 256, ("tri", 0)), (k0 + 3, 256, 256, 256, ("z", 256))]
    )
    return groups


def _emit(nc, tile, mybir, tc, ctx, aps, loop_reps=0):
    F32 = mybir.dt.float32
    F32R = mybir.dt.float32r
    EXP = mybir.ActivationFunctionType.Exp
    xT, wqk, wvp, wp, consts, out = aps
    CC = C // 128  # 6 contraction chunks for the projections

    wpool = ctx.enter_context(tc.tile_pool(name="w", bufs=1))
    qkvpool = ctx.enter_context(tc.tile_pool(name="qkv", bufs=1))
    xpool = ctx.enter_context(tc.tile_pool(name="x", bufs=4))
    ptpool = ctx.enter_context(tc.tile_pool(name="pt", bufs=6))
    atpool = ctx.enter_context(tc.tile_pool(name="at", bufs=2))
    opool = ctx.enter_context(tc.tile_pool(name="osb", bufs=4))
    rpool = ctx.enter_context(tc.tile_pool(name="r", bufs=6))
    stp = ctx.enter_context(tc.tile_pool(name="stp", bufs=2, space="PSUM"))
    osp = ctx.enter_context(tc.tile_pool(name="osp", bufs=4, space="PSUM"))

    # ---- constants and weights ----
    # tri [0:128], Z [128:384], M4 [544:1056] = [zeros(384) | tri(128)]
    masks_sb = wpool.tile([128, 1056], F32)
    nc.sync.dma_start(masks_sb[:], consts[:, 0:1056])
    ones64 = wpool.tile([1, 64], F32R)
    nc.sync.dma_start(ones64[:], consts[0:1, 384:448].bitcast(F32R))
    wqk_sb = wpool.tile([128, CC * 3 * 128], F32R)
    nc.sync.dma_start(
        wqk_sb[:].rearrange("p (a g m) -> p a g m", a=CC, g=3),
        wqk.bitcast(F32R).rearrange("(a p) g m -> p a g m", p=128),
    )
    wvp_sb = wpool.tile([128, CC * 256], F32R)
    nc.sync.dma_start(
        wvp_sb[:].rearrange("p (a n) -> p a n", a=CC),
        wvp.bitcast(F32R).rearrange("(a p) n -> p a n", p=128),
    )
    wp_sb = wpool.tile([64, 3 * C], F32R)
    nc.sync.dma_start(
        wp_sb[:].rearrange("p (g n) -> p g n", g=3),
        wp.bitcast(F32R).rearrange("g p n -> p g n"),
    )

    # ---- qkv storage ----
    # qkT01: [0:T] = qT (h0 rows 0-63, h1 rows 64-127), [T:2T] = kT
    qkT01 = qkvpool.tile([128, 2 * T], F32R)
    # qk2: rows 0-63 only: [0:T] = qT_h2, [T:2T] = kT_h2
    qk2 = qkvpool.tile([64, 2 * T], F32R)
    vbig = qkvpool.tile([128, (T // KT) * VSTRIDE], F32R)
    vbig3 = vbig[:].rearrange("p (t c) -> p t c", c=VSTRIDE)
    # ones columns of vbig (col 65h+64 per k-tile), one DMA per head
    for h in range(3):
        nc.sync.dma_start(
            vbig3[:, :, 65 * h + 64 : 65 * h + 65],
            consts[:, 448 + 32 * h : 448 + 32 * (h + 1)]
            .bitcast(F32R)
            .rearrange("p (t u) -> p t u", u=1),
        )

    if loop_reps:
        loop_cm = tc.For_i(0, loop_reps, 1)
        loop_cm.__enter__()

    def fetch_x(tb):
        t0 = tb * QB
        xh = []
        for half in range(2):
            xt = xpool.tile([128, 3 * QB], F32R, tag="xt")
            nc.sync.dma_start(
                xt[:].rearrange("p (a t) -> p a t", a=3),
                xT[384 * half : 384 * (half + 1), t0 : t0 + QB]
                .bitcast(F32R)
                .rearrange("(a p) t -> p a t", p=128),
            )
            xh.append(xt)
        return xh

    def emit_qkv(tb, xh):
        t0 = tb * QB

        def xchunk(cc):
            return xh[cc // 3][:, (cc % 3) * QB : (cc % 3 + 1) * QB]

        # pair q then pair k: full 128-col stationary operand
        for g in range(2):
            ps = stp.tile([128, 1024], F32, tag="st")
            for cc in range(CC):
                nc.tensor.matmul(
                    ps[:, 0:QB],
                    wqk_sb[:, (cc * 3 + g) * 128 : (cc * 3 + g + 1) * 128],
                    xchunk(cc),
                    start=(cc == 0),
                    stop=(cc == CC - 1),
                )
            nc.vector.tensor_copy(
                qkT01[:, g * T + t0 : g * T + t0 + QB], ps[:, 0:QB]
            )
        # head 2 q and k into one psum tile (M=64 each)
        ps2 = stp.tile([128, 1024], F32, tag="st")
        for g2 in range(2):
            for cc in range(CC):
                base = (cc * 3 + 2) * 128 + 64 * g2
                nc.tensor.matmul(
                    ps2[0:64, g2 * QB : (g2 + 1) * QB],
                    wqk_sb[:, base : base + 64],
                    xchunk(cc),
                    start=(cc == 0),
                    stop=(cc == CC - 1),
                )
        for g2 in range(2):
            nc.vector.tensor_copy(
                qk2[:, g2 * T + t0 : g2 * T + t0 + QB],
                ps2[0:64, g2 * QB : (g2 + 1) * QB],
            )
        # v: natural [t, d] layout, 4 k-tiles per tb, one psum tile
        psv = stp.tile([128, 1024], F32, tag="st")
        for tt in range(4):
            for cc in range(CC):
                nc.tensor.matmul(
                    psv[:, tt * 256 : (tt + 1) * 256],
                    xchunk(cc)[:, tt * 128 : (tt + 1) * 128],
                    wvp_sb[:, cc * 256 : (cc + 1) * 256],
                    start=(cc == 0),
                    stop=(cc == CC - 1),
                )
        for tt in range(4):
            kt = 4 * tb + tt
            dst = vbig3[:, kt, 0:195].rearrange("p (h c) -> p h c", c=65)[:, :, 0:64]
            nc.vector.tensor_copy(
                dst,
                psv[:, tt * 256 : tt * 256 + 192].rearrange(
                    "p (h d) -> p h d", h=3
                ),
            )

    # head descriptors: (row_group or None, qT ap, kT ap)
    def head_aps():
        return [
            (0, qkT01[0:64, 0:T], qkT01[0:64, T : 2 * T]),
            (1, qkT01[64:128, 0:T], qkT01[64:128, T : 2 * T]),
            (None, qk2[:, 0:T], qk2[:, T : 2 * T]),
        ]

    def emit_attend(qb, att65):
        t0 = qb * QB
        groups = _qb_entries(qb)
        last_kt = 4 * qb + 3
        o_ps = [
            osp.tile([65, QB], F32, tag="o", name=f"ops{qb}_{h}") for h in range(3)
        ]
        heads = head_aps()
        def emit_pv(ents, pts):
            for hh in range(3):
                for kt, off, w, qoff, _m in ents:
                    nc.tensor.matmul(
                        o_ps[hh][:, qoff : qoff + w],
                        vbig3[:, kt, 65 * hh : 65 * hh + 65],
                        pts[hh][:, off : off + w],
                        start=(kt == 0),
                        stop=(kt == last_kt),
                    )

        pend = None
        for gi, ents in enumerate(groups):
            tw = ents[-1][1] + ents[-1][2]  # total tile width
            pts = []
            for hh, (rg, qT_ap, kT_ap) in enumerate(heads):
                st = stp.tile([128, 1024], F32, tag="st", name=f"st{qb}_{gi}_{hh}")
                for kt, off, w, qoff, _m in ents:
                    kw = {} if rg is None else {"tile_position": (64 * rg, 0)}
                    nc.tensor.matmul(
                        st[:, off : off + w],
                        kT_ap[:, kt * KT : (kt + 1) * KT],
                        qT_ap[:, t0 + qoff : t0 + qoff + w],
                        start=True,
                        stop=True,
                        **kw,
                    )
                pt = ptpool.tile([128, 1024], F32R, tag="pt")
                nc.scalar.activation(
                    pt[:, 0:tw], st[:, 0:tw], EXP, scale=float(D) ** -0.5
                )
                for kt, off, w, qoff, m in ents:
                    if m is None:
                        continue
                    if m[0] == "tri":
                        nc.vector.tensor_mul(
                            pt[:, m[1] : m[1] + 128],
                            pt[:, m[1] : m[1] + 128],
                            masks_sb[:, 0:128],
                        )
                    elif m[0] == "z":  # first 128 cols zero, next 128 tri
                        nc.vector.tensor_mul(
                            pt[:, m[1] : m[1] + 256],
                            pt[:, m[1] : m[1] + 256],
                            masks_sb[:, 128:384],
                        )
                    else:  # m4: [zeros(128*tt) | tri] over cols [col, col+128*(tt+1))
                        _, col, tt = m
                        mw = 128 * (tt + 1)
                        nc.vector.tensor_mul(
                            pt[:, col : col + mw],
                            pt[:, col : col + mw],
                            masks_sb[:, 1056 - mw : 1056],
                        )
                pts.append(pt)
            if pend is not None:
                emit_pv(*pend)
            pend = (ents, pts)
        emit_pv(*pend)
        # drain PSUM early: copy [PV | sums] to SBUF, reciprocal from SBUF
        rss = []
        for hh in range(3):
            nc.vector.tensor_copy(
                att65[:, hh * QB : (hh + 1) * QB], o_ps[hh][:].bitcast(F32)
            )
            rs = rpool.tile([1, QB], F32R, tag="r")
            with nc.allow_low_precision(reason="f32r recip feeds f32r matmul"):
                nc.vector.reciprocal(
                    rs[:], att65[64:65, hh * QB : (hh + 1) * QB]
                )
            rss.append(rs)
        return rss

    def emit_normproj(qb, att65, rss):
        t0 = qb * QB
        # broadcast 1/rowsum across the 64 d-partitions via PE, then scale
        for hh in range(3):
            bc = stp.tile([128, 1024], F32, tag="st", name=f"bc{qb}_{hh}")
            nc.tensor.matmul(
                bc[0:64, 0:QB], ones64[:], rss[hh][:], start=True, stop=True
            )
            asl = att65[0:64, hh * QB : (hh + 1) * QB]
            nc.vector.tensor_mul(asl, asl, bc[0:64, 0:QB])
        # output projection for this query block
        for tt in range(4):
            pps = stp.tile([128, 1024], F32, tag="st", name=f"pp{qb}_{tt}")
            for j in range(2):
                # bank-aligned: j=0 -> cols [0:384], j=1 -> cols [512:896]
                for h in range(3):
                    nc.tensor.matmul(
                        pps[:, j * 512 : j * 512 + 384],
                        att65[0:64, h * QB + tt * 128 : h * QB + (tt + 1) * 128],
                        wp_sb[:, h * C + 384 * j : h * C + 384 * (j + 1)],
                        start=(h == 0),
                        stop=(h == 2),
                    )
            osb = opool.tile([128, C], F32, tag="osb")
            nc.vector.tensor_copy(osb[:, 0:384], pps[:, 0:384])
            nc.vector.tensor_copy(osb[:, 384:768], pps[:, 512:896])
            nc.sync.dma_start(out[t0 + tt * 128 : t0 + (tt + 1) * 128, :], osb[:])

    prev = None
    xh = fetch_x(0)
    for tb in range(NQB):
        if tb + 1 < NQB:
            xh_next = fetch_x(tb + 1)
        emit_qkv(tb, xh)
        xh = xh_next
        att65 = atpool.tile([65, 3 * QB], F32R, tag="att")
        rss = emit_attend(tb, att65)
        if prev is not None:
            emit_normproj(*prev)
        prev = (tb, att65, rss)
    emit_normproj(*prev)

    if loop_reps:
        loop_cm.__exit__(None, None, None)


def _build(loop_reps=0):
    import concourse.bass as bass  # noqa: F401
    import concourse.tile as tile
    import concourse.mybir as mybir
    from concourse import bacc
    from contextlib import ExitStack

    F32 = mybir.dt.float32
    nc = bacc.Bacc()
    xT = nc.dram_tensor("xT", [C, T], F32, kind="ExternalInput").ap()
    wqk = nc.dram_tensor("wqk", [C, 3, 128], F32, kind="ExternalInput").ap()
    wvp = nc.dram_tensor("wvp", [C, 256], F32, kind="ExternalInput").ap()
    wp = nc.dram_tensor("wp", [3, 64, C], F32, kind="ExternalInput").ap()
    consts = nc.dram_tensor("consts", [128, 1056], F32, kind="ExternalInput").ap()
    out = nc.dram_tensor("out", [T, C], F32, kind="ExternalOutput").ap()

    with tile.TileContext(nc) as tc, ExitStack() as ctx:
        _emit(nc, tile, mybir, tc, ctx, (xT, wqk, wvp, wp, consts, out), loop_reps)
    nc.compile()
    return nc


def _consts_np():
    consts = np.zeros((128, 1056), np.float32)
    p = np.arange(128)[:, None]
    f = np.arange(128)[None, :]
    tri = (f >= p).astype(np.float32)  # ST[k, q]: visible iff q >= k
    consts[:, 0:128] = tri
    consts[:, 256:384] = tri  # Z mask: [128:256] stays zero
    consts[:, 384:544] = 1.0
    consts[:, 928:1056] = tri  # M4: [544:928] stays zero
    return consts


def _shard_inputs(x, Wqkv, Wproj):
    consts = _consts_np()
    in_maps = []
    for c in range(NCORES):
        b = c // 4
        hs = [3 * (c % 4) + j for j in range(HPC)]
        wqk = np.zeros((C, 3, 128), np.float32)
        wqk[:, 0, 0:64] = Wqkv[:, (0 * H + hs[0]) * D : (0 * H + hs[0] + 1) * D]
        wqk[:, 0, 64:128] = Wqkv[:, (0 * H + hs[1]) * D : (0 * H + hs[1] + 1) * D]
        wqk[:, 1, 0:64] = Wqkv[:, (1 * H + hs[0]) * D : (1 * H + hs[0] + 1) * D]
        wqk[:, 1, 64:128] = Wqkv[:, (1 * H + hs[1]) * D : (1 * H + hs[1] + 1) * D]
        wqk[:, 2, 0:64] = Wqkv[:, (0 * H + hs[2]) * D : (0 * H + hs[2] + 1) * D]
        wqk[:, 2, 64:128] = Wqkv[:, (1 * H + hs[2]) * D : (1 * H + hs[2] + 1) * D]
        wvp = np.zeros((C, 256), np.float32)
        for j, h in enumerate(hs):
            wvp[:, j * 64 : (j + 1) * 64] = Wqkv[
                :, (2 * H + h) * D : (2 * H + h + 1) * D
            ]
        wp = np.stack([Wproj[h * D : (h + 1) * D, :] for h in hs]).astype(np.float32)
        in_maps.append(
            {
                "xT": np.ascontiguousarray(x[b].T),
                "wqk": wqk,
                "wvp": wvp,
                "wp": wp,
                "consts": consts,
            }
        )
    return in_maps


TRACE_DIR = None  # set by test.py to capture a profiled run
LAST_EXEC_NS = None


def kernel(x, Wqkv, Wproj, bproj):
    global LAST_EXEC_NS
    from concourse.bass_utils import run_bass_kernel_spmd

    x = np.asarray(x, np.float32)
    Wqkv = np.asarray(Wqkv, np.float32)
    Wproj = np.asarray(Wproj, np.float32)
    bproj = np.asarray(bproj, np.float32)

    if "nc" not in _COMPILED:
        _COMPILED["nc"] = _build()
    nc = _COMPILED["nc"]

    in_maps = _shard_inputs(x, Wqkv, Wproj)
    kw = {}
    if TRACE_DIR:
        kw = dict(trace=True, tmpdir=TRACE_DIR)
    r = run_bass_kernel_spmd(nc, in_maps, list(range(NCORES)), **kw)
    res = r.results
    LAST_EXEC_NS = r.exec_time_ns
    out = np.zeros((B, T, C), np.float32)
    for c in range(NCORES):
        out[c // 4] += res[c]["out"]
    out += bproj[None, None, :]
    return out


# revision 25
# speedup vs baseline: 1.7884x; 1.7884x over previous
"""Causal self-attention (B=2, T=4096, C=768, H=12, D=64) on 8 trn2 cores.

Sharding: (B, H) -> 24 (batch, head) pairs, 3 heads per core.
Core c handles batch b = c // 4 and heads 3*(c%4) .. 3*(c%4)+2.
Each core computes the qkv projection for its heads, flash-style causal
attention (no running max; unnormalized P = exp(s/8), row sums via an
appended ones-column on V), and its partial output projection
(contraction over its 192 attn-output channels). The host sums the 4
partials per batch and adds the bias.

v3: bf16 datapath (q/k/v/P/att/weights; accumulation stays fp32 in
PSUM) which also enables the PE's fast-weight-load on 128-col
stationaries. Software-pipelined emission keeps the PE warm (HAM
K=8/8): per block tb we emit qkv(tb), attention(tb) (PV matmuls one
k-group behind scores so exp latency is hidden), normalization (cheap
reciprocal_approx_fast + PE broadcast + fused scale-copy), then the
output projection of tb-1 whose results DMA straight from PSUM.
Diagonal 512-blocks are trimmed at 128 granularity.

Per-core layouts:
  xT   [768, 4096]  bf16  x[b].T (tokens in the free dim)
  wqk  [768, 3, 128] bf16 col groups: [Wq_h0|Wq_h1], [Wk_h0|Wk_h1], [Wq_h2|Wk_h2]
  wvp  [768, 256]   bf16  [Wv_h0 Wv_h1 Wv_h2 0]
  wp   [3, 64, 768] bf16  Wproj row chunk per head
  constsb [128, 736] bf16 tri mask, M4 mask, v-ones
  onesf [1, 64]     f32   ones row for the rowsum broadcast matmul
Heads 0/1 are row-paired on the PE (head 0 in partitions 0-63, head 1 in
64-127) for the D=64-contraction score matmuls; head 2 runs solo in
partitions 0-63. Scores are computed transposed (ST[k, q]) so the PV
matmul contracts k on the partition dim with V in natural [t, d] layout.
"""

import numpy as np

B, T, C, H, D = 2, 4096, 768, 12, 64
HPC = 3          # heads per core
NCORES = 8
QB = 512         # query block (psum bank width in fp32)
NQB = T // QB    # 8
KT = 128         # key tile
VSTRIDE = 200    # per-k-tile column stride in vbig (3*65 used + 5 pad)

_COMPILED = {}
TRIM = True
DEBUG_DUMP = False

# constsb column layout (bf16)
CB_TRI = 0       # [0:128]   tri: keep iff q >= k
CB_M4 = 128      # [128:640] [zeros(384) | tri(128)]
CB_VONES = 640   # [640:736] 32 ones-cols per head
CB_W = 736


def _qb_entries(qb):
    """Per-group score/PV work items for query block qb.

    Each group is a list of (ktile, col_off, width, q_off, mask) where
    mask is None, ("tri", col) or ("m4", col, tt). col_off is the
    column offset inside the group's st/pt tile; q_off the query offset
    inside the 512-query block.
    """
    k0 = 4 * qb
    groups = []
    if not TRIM:
        for g in range(2 * qb + 2):
            ents = []
            for i in range(2):
                kt = 2 * g + i
                m = ("m4", 512 * i, kt - k0) if kt >= k0 else None
                ents.append((kt, 512 * i, 512, 0, m))
            groups.append(ents)
        return groups
    for g in range(2 * qb):
        groups.append([(2 * g, 0, 512, 0, None), (2 * g + 1, 512, 512, 0, None)])
    groups.append(
        [(k0, 0, 512, 0, ("tri", 0)), (k0 + 1, 512, 384, 128, ("tri", 512))]
    )
    groups.append(
        [(k0 + 2, 0, 256, 256, ("tri", 0)), (k0 + 3, 256, 128, 384, ("tri", 256))]
    )
    return groups


def _emit(nc, tile, mybir, tc, ctx, aps, loop_reps=0):
    F32 = mybir.dt.float32
    F32R = mybir.dt.float32r
    BF16 = mybir.dt.bfloat16
    EXP = mybir.ActivationFunctionType.Exp
    xT, wqk, wvp, wp, constsb, onesf, out = aps[:7]
    CC = C // 128  # 6 contraction chunks for the projections

    wpool = ctx.enter_context(tc.tile_pool(name="w", bufs=1))
    qkvpool = ctx.enter_context(tc.tile_pool(name="qkv", bufs=1))
    xpool = ctx.enter_context(tc.tile_pool(name="x", bufs=4))
    ptpool = ctx.enter_context(tc.tile_pool(name="pt", bufs=6))
    atpool = ctx.enter_context(tc.tile_pool(name="at", bufs=2))
    opool = ctx.enter_context(tc.tile_pool(name="osb", bufs=4))
    rpool = ctx.enter_context(tc.tile_pool(name="r", bufs=6))
    stp = ctx.enter_context(tc.tile_pool(name="stp", bufs=2, space="PSUM"))
    osp = ctx.enter_context(tc.tile_pool(name="osp", bufs=4, space="PSUM"))

    # ---- constants and weights ----
    masks_sb = wpool.tile([128, CB_W], BF16)
    nc.sync.dma_start(masks_sb[:], constsb[:, :])
    ones64 = wpool.tile([1, 64], F32R)
    nc.sync.dma_start(ones64[:], onesf.bitcast(F32R))
    wqk_sb = wpool.tile([128, CC * 3 * 128], BF16)
    nc.sync.dma_start(
        wqk_sb[:].rearrange("p (a g m) -> p a g m", a=CC, g=3),
        wqk.rearrange("(a p) g m -> p a g m", p=128),
    )
    wvp_sb = wpool.tile([128, CC * 256], BF16)
    nc.sync.dma_start(
        wvp_sb[:].rearrange("p (a n) -> p a n", a=CC),
        wvp.rearrange("(a p) n -> p a n", p=128),
    )
    wp_sb = wpool.tile([64, 3 * C], BF16)
    nc.sync.dma_start(
        wp_sb[:].rearrange("p (g n) -> p g n", g=3),
        wp.rearrange("g p n -> p g n"),
    )

    # ---- qkv storage ----
    # qkT01: [0:T] = qT (h0 rows 0-63, h1 rows 64-127), [T:2T] = kT
    qkT01 = qkvpool.tile([128, 2 * T], BF16)
    # qk2: rows 0-63 only: [0:T] = qT_h2, [T:2T] = kT_h2
    qk2 = qkvpool.tile([64, 2 * T], BF16)
    vbig = qkvpool.tile([128, (T // KT) * VSTRIDE], BF16)
    vbig3 = vbig[:].rearrange("p (t c) -> p t c", c=VSTRIDE)
    # ones columns of vbig (col 65h+64 per k-tile), one DMA per head
    for h in range(3):
        nc.sync.dma_start(
            vbig3[:, :, 65 * h + 64 : 65 * h + 65],
            constsb[:, CB_VONES + 32 * h : CB_VONES + 32 * (h + 1)].rearrange(
                "p (t u) -> p t u", u=1
            ),
        )

    if loop_reps:
        loop_cm = tc.For_i(0, loop_reps, 1)
        loop_cm.__enter__()

    def fetch_x(tb):
        t0 = tb * QB
        xh = []
        for half in range(2):
            xt = xpool.tile([128, 3 * QB], BF16, tag="xt")
            nc.sync.dma_start(
                xt[:].rearrange("p (a t) -> p a t", a=3),
                xT[384 * half : 384 * (half + 1), t0 : t0 + QB].rearrange(
                    "(a p) t -> p a t", p=128
                ),
            )
            xh.append(xt)
        return xh

    def emit_qkv(tb, xh):
        t0 = tb * QB

        def xchunk(cc):
            return xh[cc // 3][:, (cc % 3) * QB : (cc % 3 + 1) * QB]

        # pair q then pair k: full 128-col stationary operand
        for g in range(2):
            ps = stp.tile([128, 1024], F32, tag="st")
            for cc in range(CC):
                nc.tensor.matmul(
                    ps[:, 0:QB],
                    wqk_sb[:, (cc * 3 + g) * 128 : (cc * 3 + g + 1) * 128],
                    xchunk(cc),
                    start=(cc == 0),
                    stop=(cc == CC - 1),
                )
            nc.vector.tensor_copy(
                qkT01[:, g * T + t0 : g * T + t0 + QB], ps[:, 0:QB]
            )
        # head 2 q and k into one psum tile (M=64 each)
        ps2 = stp.tile([128, 1024], F32, tag="st")
        for g2 in range(2):
            for cc in range(CC):
                base = (cc * 3 + 2) * 128 + 64 * g2
                nc.tensor.matmul(
                    ps2[0:64, g2 * QB : (g2 + 1) * QB],
                    wqk_sb[:, base : base + 64],
                    xchunk(cc),
                    start=(cc == 0),
                    stop=(cc == CC - 1),
                )
        for g2 in range(2):
            nc.vector.tensor_copy(
                qk2[:, g2 * T + t0 : g2 * T + t0 + QB],
                ps2[0:64, g2 * QB : (g2 + 1) * QB],
            )
        # v: natural [t, d] layout, 4 k-tiles per tb, one psum tile
        psv = stp.tile([128, 1024], F32, tag="st")
        for tt in range(4):
            for cc in range(CC):
                nc.tensor.matmul(
                    psv[:, tt * 256 : (tt + 1) * 256],
                    xchunk(cc)[:, tt * 128 : (tt + 1) * 128],
                    wvp_sb[:, cc * 256 : (cc + 1) * 256],
                    start=(cc == 0),
                    stop=(cc == CC - 1),
                )
        for tt in range(4):
            kt = 4 * tb + tt
            dst = vbig3[:, kt, 0:195].rearrange("p (h c) -> p h c", c=65)[:, :, 0:64]
            nc.vector.tensor_copy(
                dst,
                psv[:, tt * 256 : tt * 256 + 192].rearrange(
                    "p (h d) -> p h d", h=3
                ),
            )

    # head descriptors: (row_group or None, qT ap, kT ap)
    def head_aps():
        return [
            (0, qkT01[0:64, 0:T], qkT01[0:64, T : 2 * T]),
            (1, qkT01[64:128, 0:T], qkT01[64:128, T : 2 * T]),
            (None, qk2[:, 0:T], qk2[:, T : 2 * T]),
        ]

    def emit_attend(qb, att65):
        t0 = qb * QB
        groups = _qb_entries(qb)
        last_kt = 4 * qb + 3
        o_ps = [
            osp.tile([65, QB], F32, tag="o", name=f"ops{qb}_{h}") for h in range(3)
        ]
        heads = head_aps()

        def emit_pv(ents, pts):
            for hh in range(3):
                for kt, off, w, qoff, _m in ents:
                    nc.tensor.matmul(
                        o_ps[hh][:, qoff : qoff + w],
                        vbig3[:, kt, 65 * hh : 65 * hh + 65],
                        pts[hh][:, off : off + w],
                        start=(kt == 0),
                        stop=(kt == last_kt),
                    )

        pend = None
        for gi, ents in enumerate(groups):
            tw = ents[-1][1] + ents[-1][2]  # total tile width
            pts = []
            for hh, (rg, qT_ap, kT_ap) in enumerate(heads):
                st = stp.tile([128, 1024], F32, tag="st", name=f"st{qb}_{gi}_{hh}")
                for kt, off, w, qoff, _m in ents:
                    kw = {} if rg is None else {"tile_position": (64 * rg, 0)}
                    nc.tensor.matmul(
                        st[:, off : off + w],
                        kT_ap[:, kt * KT : (kt + 1) * KT],
                        qT_ap[:, t0 + qoff : t0 + qoff + w],
                        start=True,
                        stop=True,
                        **kw,
                    )
                pt = ptpool.tile([128, 1024], BF16, tag="pt")
                nc.scalar.activation(
                    pt[:, 0:tw], st[:, 0:tw], EXP, scale=float(D) ** -0.5
                )
                for kt, off, w, qoff, m in ents:
                    if m is None:
                        continue
                    if m[0] == "tri":
                        nc.vector.tensor_mul(
                            pt[:, m[1] : m[1] + 128],
                            pt[:, m[1] : m[1] + 128],
                            masks_sb[:, CB_TRI : CB_TRI + 128],
                        )
                    else:  # m4: [zeros(128*tt) | tri] over cols [col, col+128*(tt+1))
                        _, col, tt = m
                        mw = 128 * (tt + 1)
                        nc.vector.tensor_mul(
                            pt[:, col : col + mw],
                            pt[:, col : col + mw],
                            masks_sb[:, CB_M4 + 512 - mw : CB_M4 + 512],
                        )
                pts.append(pt)
            if pend is not None:
                emit_pv(*pend)
            pend = (ents, pts)
        emit_pv(*pend)

        return o_ps

    def emit_norm(qb, att65, o_ps):
        # normalize: 1/rowsum (cheap approx from PSUM row 64), round to f32r,
        # PE broadcast, fused scale+cast into bf16 att
        rss = []
        for hh in range(3):
            ssb = rpool.tile([1, QB], F32, tag="s")
            nc.vector.tensor_copy(ssb[:], o_ps[hh][64:65, :])
            rs = rpool.tile([1, QB], F32, tag="r")
            nc.vector.reciprocal_approx_fast(rs[:], ssb[:])
            rsr = rpool.tile([1, QB], F32R, tag="rr")
            with nc.allow_low_precision(reason="f32r recip feeds f32r matmul"):
                nc.vector.tensor_copy(rsr[:], rs[:])
            rss.append(rsr)
        for hh in range(3):
            asl = att65[0:64, hh * QB : (hh + 1) * QB]
            nc.vector.tensor_copy(asl, o_ps[hh][0:64, :])
            bc = stp.tile([128, 1024], F32, tag="st", name=f"bc{qb}_{hh}")
            nc.tensor.matmul(
                bc[0:64, 0:QB], ones64[:], rss[hh][:], start=True, stop=True
            )
            nc.vector.tensor_mul(asl, asl, bc[0:64, 0:QB])

    def emit_proj(qb, att65):
        t0 = qb * QB
        for tt in range(4):
            pps = stp.tile([128, 1024], F32, tag="st", name=f"pp{qb}_{tt}")
            for j in range(2):
                # bank-aligned: j=0 -> cols [0:384], j=1 -> cols [512:896]
                for h in range(3):
                    nc.tensor.matmul(
                        pps[:, j * 512 : j * 512 + 384],
                        att65[0:64, h * QB + tt * 128 : h * QB + (tt + 1) * 128],
                        wp_sb[:, h * C + 384 * j : h * C + 384 * (j + 1)],
                        start=(h == 0),
                        stop=(h == 2),
                    )
            osb = opool.tile([128, C], F32, tag="osb")
            nc.vector.tensor_copy(osb[:, 0:384], pps[:, 0:384])
            nc.vector.tensor_copy(osb[:, 384:768], pps[:, 512:896])
            r0 = t0 + tt * 128
            nc.sync.dma_start(out[r0 : r0 + 128, :], osb[:])

    prev = None
    xh = fetch_x(0)
    for tb in range(NQB):
        if tb + 1 < NQB:
            xh_next = fetch_x(tb + 1)
        emit_qkv(tb, xh)
        xh = xh_next
        att65 = atpool.tile([64, 3 * QB], BF16, tag="att")
        o_ps = emit_attend(tb, att65)
        if prev is not None:
            emit_proj(*prev)
        emit_norm(tb, att65, o_ps)
        prev = (tb, att65)
    emit_proj(*prev)

    if DEBUG_DUMP:
        dbg = aps[-1]  # extra dram tensor appended by _build
        nc.sync.dma_start(dbg[:, 0:512], qkT01[:, 0:512])
        nc.sync.dma_start(dbg[:, 512:1024], qkT01[:, T : T + 512])
        nc.sync.dma_start(dbg[0:64, 1024:1536], qk2[:, 0:512])
        nc.sync.dma_start(dbg[0:64, 1536:2048], qk2[:, T : T + 512])
        nc.sync.dma_start(dbg[:, 2048:2243], vbig3[:, 0, 0:195])
        nc.sync.dma_start(dbg[0:64, 2304:3840], prev[1][:])

    if loop_reps:
        loop_cm.__exit__(None, None, None)


def _build(loop_reps=0):
    import concourse.bass as bass  # noqa: F401
    import concourse.tile as tile
    import concourse.mybir as mybir
    from concourse import bacc
    from contextlib import ExitStack

    F32 = mybir.dt.float32
    BF16 = mybir.dt.bfloat16
    nc = bacc.Bacc()
    xT = nc.dram_tensor("xT", [C, T], BF16, kind="ExternalInput").ap()
    wqk = nc.dram_tensor("wqk", [C, 3, 128], BF16, kind="ExternalInput").ap()
    wvp = nc.dram_tensor("wvp", [C, 256], BF16, kind="ExternalInput").ap()
    wp = nc.dram_tensor("wp", [3, 64, C], BF16, kind="ExternalInput").ap()
    constsb = nc.dram_tensor("constsb", [128, CB_W], BF16, kind="ExternalInput").ap()
    onesf = nc.dram_tensor("onesf", [1, 64], F32, kind="ExternalInput").ap()
    out = nc.dram_tensor("out", [T, C], F32, kind="ExternalOutput").ap()

    aps = (xT, wqk, wvp, wp, constsb, onesf, out)
    if DEBUG_DUMP:
        dbg = nc.dram_tensor("dbg", [128, 3840], BF16, kind="ExternalOutput").ap()
        aps = aps + (dbg,)
    with tile.TileContext(nc) as tc, ExitStack() as ctx:
        _emit(nc, tile, mybir, tc, ctx, aps, loop_reps)
    nc.compile()
    return nc


def _consts_np():
    import ml_dtypes

    consts = np.zeros((128, CB_W), np.float32)
    p = np.arange(128)[:, None]
    f = np.arange(128)[None, :]
    tri = (f >= p).astype(np.float32)  # ST[k, q]: visible iff q >= k
    consts[:, CB_TRI : CB_TRI + 128] = tri
    consts[:, CB_M4 + 384 : CB_M4 + 512] = tri  # M4: leading 384 stay zero
    consts[:, CB_VONES:CB_W] = 1.0
    return consts.astype(ml_dtypes.bfloat16)


def _shard_inputs(x, Wqkv, Wproj):
    import ml_dtypes

    bf = ml_dtypes.bfloat16
    constsb = _consts_np()
    onesf = np.ones((1, 64), np.float32)
    in_maps = []
    for c in range(NCORES):
        b = c // 4
        hs = [3 * (c % 4) + j for j in range(HPC)]
        wqk = np.zeros((C, 3, 128), np.float32)
        wqk[:, 0, 0:64] = Wqkv[:, (0 * H + hs[0]) * D : (0 * H + hs[0] + 1) * D]
        wqk[:, 0, 64:128] = Wqkv[:, (0 * H + hs[1]) * D : (0 * H + hs[1] + 1) * D]
        wqk[:, 1, 0:64] = Wqkv[:, (1 * H + hs[0]) * D : (1 * H + hs[0] + 1) * D]
        wqk[:, 1, 64:128] = Wqkv[:, (1 * H + hs[1]) * D : (1 * H + hs[1] + 1) * D]
        wqk[:, 2, 0:64] = Wqkv[:, (0 * H + hs[2]) * D : (0 * H + hs[2] + 1) * D]
        wqk[:, 2, 64:128] = Wqkv[:, (1 * H + hs[2]) * D : (1 * H + hs[2] + 1) * D]
        wvp = np.zeros((C, 256), np.float32)
        for j, h in enumerate(hs):
            wvp[:, j * 64 : (j + 1) * 64] = Wqkv[
                :, (2 * H + h) * D : (2 * H + h + 1) * D
            ]
        wp = np.stack([Wproj[h * D : (h + 1) * D, :] for h in hs]).astype(np.float32)
        in_maps.append(
            {
                "xT": np.ascontiguousarray(x[b].T).astype(bf),
                "wqk": wqk.astype(bf),
                "wvp": wvp.astype(bf),
                "wp": wp.astype(bf),
                "constsb": constsb,
                "onesf": onesf,
            }
        )
    return in_maps


TRACE_DIR = None  # set by test.py to capture a profiled run
LAST_EXEC_NS = None


def kernel(x, Wqkv, Wproj, bproj):
    global LAST_EXEC_NS
    from concourse.bass_utils import run_bass_kernel_spmd

    x = np.asarray(x, np.float32)
    Wqkv = np.asarray(Wqkv, np.float32)
    Wproj = np.asarray(Wproj, np.float32)
    bproj = np.asarray(bproj, np.float32)

    if "nc" not in _COMPILED:
        _COMPILED["nc"] = _build()
    nc = _COMPILED["nc"]

    in_maps = _shard_inputs(x, Wqkv, Wproj)
    kw = {}
    if TRACE_DIR:
        kw = dict(trace=True, tmpdir=TRACE_DIR)
    r = run_bass_kernel_spmd(nc, in_maps, list(range(NCORES)), **kw)
    res = r.results
    LAST_EXEC_NS = r.exec_time_ns
    out = np.zeros((B, T, C), np.float32)
    for c in range(NCORES):
        out[c // 4] += res[c]["out"]
    out += bproj[None, None, :]
    return out
